# revision 23
# baseline (speedup 1.0000x reference)
"""MultiHeadAttention on 8 TRN2 NeuronCores: DP=2 (batch) x TP=4 (heads).

Shapes (hardcoded): x [4, 2048, 1024], 16 heads x 64 dim, causal.
Per core: 2 batches, 4 heads. Device computes QKV, causal softmax
attention (with an augmented ones-column in V to produce softmax
denominators), and a partial out-projection over its 256 v-dims.
Host sums partials over the 4 TP cores (all-reduce) and adds bo.

v2: score tiles paired into [128,1024] PSUM groups so one ACT exp
covers two k-tiles; av matmuls deferred one group so PE overlaps the
exp; causal masks on Pool; out-projection interleaved per q-chunk.

All SBUF tiles consumed by the PE are float32r (the BIR verifier
requires producers of fp32r-matmul operands to emit f32r).
"""

import os
from contextlib import ExitStack

import numpy as np

import concourse.mybir as mybir
import concourse.tile as tile
from concourse import bacc

B, T, D = 4, 2048, 1024
H, HD = 16, 64
DP, TP = 2, 4
NB = B // DP          # batches per core
NH = H // TP          # heads per core
HV = HD + 1           # head dim + ones column
VA = NH * HV          # 260 augmented v columns
QH = NH * HD          # 256 q/k columns per core
SCALE = 1.0 / 8.0     # 1/sqrt(HD)

LAST_EXEC_NS = None
_NC = None


def build_nc():
    f32 = mybir.dt.float32
    fr = mybir.dt.float32r
    Exp = mybir.ActivationFunctionType.Exp
    Copy = mybir.ActivationFunctionType.Copy

    nc = bacc.Bacc(trn_type="TRN2", target_bir_lowering=False, debug=False)
    xt = nc.declare_dram_parameter("xt", [NB * D, T], fr, isOutput=False)
    wq = nc.declare_dram_parameter("wq", [D, QH], fr, isOutput=False)
    wk = nc.declare_dram_parameter("wk", [D, QH], fr, isOutput=False)
    wv = nc.declare_dram_parameter("wv", [D, VA], fr, isOutput=False)
    wo = nc.declare_dram_parameter("wo", [QH, D], fr, isOutput=False)
    bq = nc.declare_dram_parameter("bq", [QH, 1], f32, isOutput=False)
    bk = nc.declare_dram_parameter("bk", [QH, 1], f32, isOutput=False)
    bv = nc.declare_dram_parameter("bv", [1, VA], fr, isOutput=False)
    on = nc.declare_dram_parameter("on", [1, 128], fr, isOutput=False)
    tr = nc.declare_dram_parameter("tr", [128, 128], fr, isOutput=False)
    y = nc.declare_dram_parameter("y", [NB * T, D], f32, isOutput=True)

    with tile.TileContext(nc) as tc, ExitStack() as ctx:
        cpool = ctx.enter_context(tc.tile_pool(name="const", bufs=1))
        ppool = ctx.enter_context(tc.tile_pool(name="persist", bufs=1))
        xpool = ctx.enter_context(tc.tile_pool(name="xin", bufs=2))
        epool = ctx.enter_context(tc.tile_pool(name="escores", bufs=3))
        ypool = ctx.enter_context(tc.tile_pool(name="yout", bufs=2))
        rpool = ctx.enter_context(tc.tile_pool(name="recip", bufs=2))
        psum = ctx.enter_context(tc.tile_pool(name="ps", bufs=1, space="PSUM"))

        # ---- load constants (ordered by first use in the schedule) ----
        engs3 = (nc.sync, nc.gpsimd, nc.scalar)
        wq_t, wk_t, wv_t = [], [], []
        for dc in range(8):
            tq = cpool.tile([128, QH], fr, tag=f"wq{dc}")
            engs3[dc % 3].dma_start(tq[:], wq[128 * dc:128 * (dc + 1), :])
            wq_t.append(tq)
        bq_t, bk_t = [], []
        for p in range(2):
            tb = cpool.tile([128, 1], f32, tag=f"bq{p}")
            nc.gpsimd.dma_start(tb[:], bq[128 * p:128 * (p + 1), :])
            bq_t.append(tb)
            tb = cpool.tile([128, 1], f32, tag=f"bk{p}")
            nc.gpsimd.dma_start(tb[:], bk[128 * p:128 * (p + 1), :])
            bk_t.append(tb)

        def load_late_consts():
            for dc in range(8):
                tk = cpool.tile([128, QH], fr, tag=f"wk{dc}", name=f"wk{dc}")
                engs3[dc % 3].dma_start(
                    tk[:], wk[128 * dc:128 * (dc + 1), :])
                wk_t.append(tk)
            for dc in range(8):
                tv = cpool.tile([128, VA], fr, tag=f"wv{dc}", name=f"wv{dc}")
                engs3[dc % 3].dma_start(
                    tv[:], wv[128 * dc:128 * (dc + 1), :])
                wv_t.append(tv)
            bv_l = cpool.tile([1, VA], fr, tag="bv")
            nc.gpsimd.dma_start(bv_l[:], bv[:, :])
            on_l = cpool.tile([1, 128], fr, tag="on")
            nc.gpsimd.dma_start(on_l[:], on[:, :])
            tr_l = cpool.tile([128, 128], fr, tag="tr")
            nc.sync.dma_start(tr_l[:], tr[:, :])
            # [zeros(128,128) | tril-mask]: masks the widened d3 tile
            zt_l = cpool.tile([128, 256], fr, tag="zt")
            nc.gpsimd.tensor_scalar_mul(zt_l[:, 0:128], tr_l[:], 0.0)
            nc.gpsimd.tensor_copy(zt_l[:, 128:256], tr_l[:])
            # bias row broadcast across partitions for the V copy-add
            bvb_l = cpool.tile([128, VA], fr, tag="bvb")
            nc.gpsimd.partition_broadcast(bvb_l[:], bv_l[:], channels=128)
            wo_l = []
            for kc in range(2):
                tw = cpool.tile([128, D], fr, tag=f"wo{kc}", name=f"wo{kc}")
                nc.scalar.dma_start(tw[:], wo[128 * kc:128 * (kc + 1), :])
                wo_l.append(tw)
            return tr_l, zt_l, bvb_l, wo_l

        # persistent tiles: qt/ot single-buffered (chunk lifetimes are
        # disjoint across batches in the pipelined schedule); kt/va
        # double-buffered by batch parity (live across a whole batch).
        qt_t = [ppool.tile([128, T], fr, tag=f"qt{p}", name=f"qt{p}")
                for p in range(2)]
        ot_t = [ppool.tile([128, T], fr, tag=f"ot{p}", name=f"ot{p}")
                for p in range(2)]
        kt_t = [[ppool.tile([128, T], fr, tag=f"kt{par}{p}",
                            name=f"kt{par}{p}")
                 for p in range(2)] for par in range(2)]
        va_t = [[ppool.tile([128, VA], fr, tag=f"va{par}{i}",
                            name=f"va{par}{i}")
                 for i in range(16)] for par in range(2)]

        def emit_loads(ci, engs=None):
            if engs is None:
                engs = (nc.sync, nc.gpsimd)
            b, j = divmod(ci, 4)
            ts = []
            for dc in range(8):
                eng = engs[dc % len(engs)]
                tx = xpool.tile([128, 512], fr, tag=f"x{dc}")
                eng.dma_start(
                    tx[:],
                    xt[b * D + 128 * dc:b * D + 128 * (dc + 1),
                       512 * j:512 * (j + 1)])
                ts.append(tx)
            return ts

        def make_closures(ci, xt_t):
            """Phase-A psum groups for chunk ci as deferred emitters."""
            b, j = divmod(ci, 4)
            par = b % 2
            cl = []

            def proj(p, w_t, dst, b_t):
                def f():
                    pp = psum.tile([128, 512], f32, tag="a", bufs=2)
                    for dc in range(8):
                        nc.tensor.matmul(
                            pp[:],
                            w_t[dc][:, 128 * p:128 * (p + 1)],
                            xt_t[dc][:],
                            start=(dc == 0), stop=(dc == 7))
                    nc.vector.tensor_scalar_add(
                        dst[:, 512 * j:512 * (j + 1)], pp[:], b_t[:])
                return f

            for p in range(2):
                cl.append(proj(p, wq_t, qt_t[p], bq_t[p]))
            for p in range(2):
                cl.append(proj(p, wk_t, kt_t[par][p], bk_t[p]))

            def vproj(tt):
                def f():
                    pv = psum.tile([128, 512], f32, tag="a", bufs=2)
                    for dc in range(8):
                        nc.tensor.matmul(
                            pv[:, 0:VA],
                            xt_t[dc][:, 128 * tt:128 * (tt + 1)],
                            wv_t[dc][:],
                            start=(dc == 0), stop=(dc == 7))
                    nc.vector.tensor_add(
                        va_t[par][4 * j + tt][:], pv[:, 0:VA], bvb_t[:])
                return f

            for tt in range(4):
                cl.append(vproj(tt))
            return cl

        def emit_unit(ci, closures):
            """B (attention) + C (out-proj) for chunk ci, interleaving
            the next chunk's phase-A groups between score groups."""
            b, j = divmod(ci, 4)
            par = b % 2
            ni = 4 * j + 4
            ng = 2 * j + 2
            # pace closures to finish ~70% through the h-loop so the
            # next unit's inputs are ready before its first groups
            stride = max(1, (NH * ng * 7) // (10 * max(1, len(closures))))
            cnt = 0
            cidx = 0
            for h in range(NH):
                hp, hr = divmod(h, 2)
                r0 = 64 * hr
                ov = psum.tile([128, 512], f32, tag="ov", bufs=2)
                pend = []
                for g in range(ng):
                    dg = g - 2 * j
                    # (i, packed col, q-col offset st, width)
                    if dg < 0:
                        subs = [(2 * g, 0, 0, 512),
                                (2 * g + 1, 512, 0, 512)]
                        ew = 1024
                    elif dg == 0:
                        subs = [(4 * j, 0, 0, 512),
                                (4 * j + 1, 512, 128, 384)]
                        ew = 896
                    else:
                        # d3 widened to N=256 (avoids fp32r N<256 4x
                        # penalty); its extra q-cols are masked by zt
                        subs = [(4 * j + 2, 0, 256, 256),
                                (4 * j + 3, 256, 256, 256)]
                        ew = 512
                    sc = psum.tile([128, 1024], f32, tag="s", bufs=2)
                    et = epool.tile([128, 1024], fr, tag="e")
                    for (i, pc, st, w) in subs:
                        nc.tensor.matmul(
                            sc[:, pc:pc + w],
                            kt_t[par][hp][r0:r0 + 64,
                                          128 * i:128 * (i + 1)],
                            qt_t[hp][r0:r0 + 64,
                                     512 * j + st:512 * (j + 1)],
                            start=True, stop=True)
                    nc.scalar.activation(
                        et[:, 0:ew], sc[:, 0:ew], Exp, scale=SCALE)
                    if dg == 0:
                        nc.gpsimd.tensor_mul(
                            et[:, 0:128], et[:, 0:128], tr_t[:])
                        nc.gpsimd.tensor_mul(
                            et[:, 512:640], et[:, 512:640], tr_t[:])
                    elif dg == 1:
                        nc.gpsimd.tensor_mul(
                            et[:, 0:128], et[:, 0:128], tr_t[:])
                        nc.gpsimd.tensor_mul(
                            et[:, 256:512], et[:, 256:512], zt_t[:])
                    for (i, pc, st, w, ep) in pend:
                        nc.tensor.matmul(
                            ov[0:HV, st:512],
                            va_t[par][i][:, HV * h:HV * (h + 1)],
                            ep[:, pc:pc + w],
                            start=(i == 0), stop=(i == ni - 1))
                    pend = [(i, pc, st, w, et) for (i, pc, st, w) in subs]
                    cnt += 1
                    if cidx < len(closures) and cnt % stride == 0:
                        closures[cidx]()
                        cidx += 1
                for (i, pc, st, w, ep) in pend:
                    nc.tensor.matmul(
                        ov[0:HV, st:512],
                        va_t[par][i][:, HV * h:HV * (h + 1)],
                        ep[:, pc:pc + w],
                        start=(i == 0), stop=(i == ni - 1))
                rt = rpool.tile([1, 512], f32, tag="r")
                nc.vector.reciprocal(rt[:], ov[64:65, :])
                bc = rpool.tile([64, 512], f32, tag="bc")
                nc.gpsimd.partition_broadcast(bc[:], rt[:], channels=64)
                nc.vector.tensor_mul(
                    ot_t[hp][r0:r0 + 64, 512 * j:512 * (j + 1)],
                    ov[0:64, :], bc[:])

            # ---- phase C for q-blocks of this chunk ----
            for qq in range(4):
                q = 4 * j + qq
                for do_ in range(2):
                    yp = psum.tile([128, 512], f32, tag="a", bufs=2)
                    for kc in range(2):
                        nc.tensor.matmul(
                            yp[:],
                            ot_t[kc][:, 128 * q:128 * (q + 1)],
                            wo_t[kc][:, 512 * do_:512 * (do_ + 1)],
                            start=(kc == 0), stop=(kc == 1))
                    yt = ypool.tile([128, 512], f32, tag="y")
                    nc.vector.tensor_copy(yt[:], yp[:])
                    nc.sync.dma_start(
                        y[b * T + 128 * q:b * T + 128 * (q + 1),
                          512 * do_:512 * (do_ + 1)], yt[:])
                    if cidx < len(closures):
                        closures[cidx]()
                        cidx += 1
            while cidx < len(closures):
                closures[cidx]()
                cidx += 1

        # ---- pipelined schedule over 8 chunks (2 batches x 4 j) ----
        xs = emit_loads(0, engs3)
        tr_t, zt_t, bvb_t, wo_t = load_late_consts()
        for f in make_closures(0, xs):
            f()
        xs = emit_loads(1, engs3)
        pending = make_closures(1, xs)
        for ci in range(8):
            if ci + 2 <= 7:
                xs = emit_loads(ci + 2)
            emit_unit(ci, pending)
            pending = make_closures(ci + 2, xs) if ci + 2 <= 7 else []

    nc.compile()
    return nc


def make_in_maps(inputs):
    x = inputs["x"].astype(np.float32)
    Wq, Wk, Wv, Wo = (inputs[k].astype(np.float32)
                      for k in ("Wq", "Wk", "Wv", "Wo"))
    bq, bk, bv = (inputs[k].astype(np.float32) for k in ("bq", "bk", "bv"))
    on = np.ones((1, 128), np.float32)
    tr = np.triu(np.ones((128, 128), np.float32))
    in_maps = []
    for c in range(8):
        dp, tp = divmod(c, TP)
        xt = np.ascontiguousarray(
            np.concatenate([x[NB * dp + bb].T for bb in range(NB)], axis=0))
        wq_c = np.ascontiguousarray(Wq[:, tp * QH:(tp + 1) * QH])
        wk_c = np.ascontiguousarray(Wk[:, tp * QH:(tp + 1) * QH])
        wv_c = np.zeros((D, VA), np.float32)
        bv_c = np.zeros((1, VA), np.float32)
        for hh in range(NH):
            g = tp * NH + hh
            wv_c[:, HV * hh:HV * hh + HD] = Wv[:, g * HD:(g + 1) * HD]
            bv_c[0, HV * hh:HV * hh + HD] = bv[g * HD:(g + 1) * HD]
            bv_c[0, HV * hh + HD] = 1.0
        wo_c = np.ascontiguousarray(Wo[tp * QH:(tp + 1) * QH, :])
        bq_c = np.ascontiguousarray(bq[tp * QH:(tp + 1) * QH].reshape(QH, 1))
        bk_c = np.ascontiguousarray(bk[tp * QH:(tp + 1) * QH].reshape(QH, 1))
        in_maps.append({
            "xt": xt, "wq": wq_c, "wk": wk_c, "wv": wv_c, "wo": wo_c,
            "bq": bq_c, "bk": bk_c, "bv": bv_c, "on": on, "tr": tr,
        })
    return in_maps


def kernel(**inputs):
    global LAST_EXEC_NS, _NC
    from concourse.bass_utils import run_bass_kernel_spmd

    if _NC is None:
        _NC = build_nc()
    in_maps = make_in_maps(inputs)
    res = run_bass_kernel_spmd(_NC, in_maps, core_ids=list(range(8)))

    bo = inputs["bo"].astype(np.float64)
    y_full = np.zeros((B, T, D), np.float64)
    for c in range(8):
        dp, tp = divmod(c, TP)
        yc = np.asarray(res.results[c]["y"]).astype(np.float64)
        for bb in range(NB):
            y_full[NB * dp + bb] += yc[bb * T:(bb + 1) * T, :]
    y_full += bo
    return y_full.astype(np.float32)
